# revision 1
# baseline (speedup 1.0000x reference)
"""AKT dense-transformer kernel for 8 TRN2 NeuronCores.

Sharding: pure data-parallel over batch (B=16 -> 2 per core). Each core runs
the full 3-block model on its 2 batch elements; no collectives needed.

Per-core design:
- Activations feature-major ([d_model partition-tiles, seq free]) so all
  projections/FFN matmuls need no activation transposes; residual stream fp32,
  matmul operands bf16 (PSUM accumulates fp32).
- Attention matrices [query i partitions, key j free]; every per-query stat
  (softmax Z via activation accum_out / scan tail, cumsum, distance mass) is a
  per-partition scalar op. Causal packing: tile (i-chunk ci) only computes
  j < 128*(ci+1).
- The AKT distance decay: p = softmax(masked s); cum = cumsum_j(p) via DVE
  tensor_tensor_scan; u = (1 - cum)*|i-j|; te = exp(-softplus(g)*sqrt(u));
  attn = softmax(masked s*te). Sqrt lives in a different ACT table set than
  Exp, so each block runs phases A(exp)/B(sqrt)/C(exp) per batch element,
  with a program-order dependency chain on transcendental ACT ops only.
- attn^T for the attn@v contraction comes from DMA-xbar block transposes
  (one 3D dma_start_transpose per (head, i-chunk), alternating SP/ACT HWDGE
  queues). q==k (the model shares Wk), so only one kq projection; 1/sqrt(DK)
  is folded into Wk host-side; all-zero biases and unit LN gains of this
  model are elided.
- LayerNorm feature-major: sums/sq-sums via PE ones-matmuls, rstd =
  exp(-0.5*ln(var+eps)) on ACT, per-token broadcast materialized by GPSIMD
  partition_broadcast, normalize on GPSIMD tensor_tensor.
- Work is spread across all five engines; out-proj/LN/FFN of one batch elem
  overlaps the attention phases of the other.
"""

import math

import ml_dtypes
import numpy as np

import concourse.bass as bass
import concourse.bacc as bacc
import concourse.mybir as mybir
from concourse.tile import TileContext
from concourse.tile_rust import add_dep_helper
from concourse.bass_utils import run_bass_kernel_spmd

F32 = mybir.dt.float32
BF16 = mybir.dt.bfloat16
AF = mybir.ActivationFunctionType
OP = mybir.AluOpType

B, S, D, H, FF, L = 16, 512, 512, 8, 2048, 3
DK = D // H          # 64
NCORES = 8
BL = B // NCORES     # 2 batch elems per core
P = 128
NCD = D // P         # 4 feature chunks
NCS = S // P         # 4 token chunks
NCF = FF // P        # 16 ff chunks
EPS = 1e-5

# block -> (mask index, has_ffn, qk input, v input)   0=mask1(j<=i) 1=mask0(j<i)
BLOCKS = [(0, True, "y", "y"), (0, False, "x", "x"), (1, True, "x", "y")]

_CACHE = {}


def _softplus(x):
    return np.logaddexp(0.0, x)


def _T(pool, shape, dtype, tag):
    return pool.tile(shape, dtype, tag=tag, name=tag)


def build_graph(repeat=1):
    nc = bacc.Bacc(None, target_bir_lowering=False)

    # register EPS as an activation-bias constant (only 0.0/1.0 pre-registered)
    _eps_t = nc.alloc_sbuf_tensor("const-eps", [P, 1], F32)
    nc.gpsimd.memset(_eps_t.ap(), EPS)
    nc.const_aps.aps[(F32, EPS)] = _eps_t.ap()
    nc.all_engine_barrier()

    # ---- DRAM parameters (per-core shapes) ----
    def inp(name, shape, dtype):
        return nc.declare_dram_parameter(name, list(shape), dtype, isOutput=False)

    xT = inp("xT", [BL, D, S], F32)          # q_embed, feature-major
    yT = inp("yT", [BL, D, S], F32)          # qa_embed, feature-major
    wk = inp("wk", [L, D, D], BF16)          # pre-scaled by 1/sqrt(sqrt(DK))... (1/8^.5)
    wv = inp("wv", [L, D, D], BF16)
    wo = inp("wo", [L, D, D], BF16)
    w1 = inp("w1", [L, D, FF], BF16)
    w2 = inp("w2", [L, FF, D], BF16)
    g2n = inp("g2n", [L, H, P], F32)         # -(softplus(gamma))^2, bcast over 128
    posn = inp("posn", [NCS, P, S], BF16)    # |i-j| per i-chunk
    maskd = inp("maskd", [2, P, P], BF16)    # binary diag mask (1=keep)
    maskinf = inp("maskinf", [2, P, P], BF16)   # additive diag mask (0 / -inf)
    ninv = inp("ninv", [2, P], F32)              # invalid count per diag row
    ident = inp("ident", [P, P], F32)
    out = nc.declare_dram_parameter("out", [BL, S, D], F32, isOutput=True)

    act_chain = []  # serialize ACT engine in program order (table-set phases)

    _TRANS = (AF.Exp, AF.Ln, AF.Sqrt)

    def act(*args, **kwargs):
        i = nc.scalar.activation(*args, **kwargs)
        func = args[2] if len(args) > 2 else kwargs.get("func")
        if func in _TRANS:
            if act_chain:
                add_dep_helper(i.ins, act_chain[-1].ins, sync=False,
                               reason="act table-set order")
            act_chain.append(i)
        return i

    with TileContext(nc) as tc:
        with (
            tc.tile_pool(name="const", bufs=1) as c_pool,
            tc.tile_pool(name="wk_p", bufs=2) as wk_pool,
            tc.tile_pool(name="wvo", bufs=1) as wvo_pool,
            tc.tile_pool(name="wff", bufs=1) as wff_pool,
            tc.tile_pool(name="act", bufs=1) as a_pool,      # residual stream f32
            tc.tile_pool(name="abf", bufs=1) as abf_pool,    # bf16 matmul copies
            tc.tile_pool(name="kqv", bufs=1) as kqv_pool,
            tc.tile_pool(name="attn", bufs=2) as at_pool,    # per-head transients
            tc.tile_pool(name="u", bufs=1) as u_pool,
            tc.tile_pool(name="hff", bufs=1) as h_pool,
            tc.tile_pool(name="small", bufs=1) as s_pool,
            tc.tile_pool(name="psA", bufs=3, space="PSUM") as psA,   # scores/s2
            tc.tile_pool(name="psB", bufs=2, space="PSUM") as psB,   # proj/ffn
            tc.tile_pool(name="psC", bufs=1, space="PSUM") as psC,   # av/stats/tr
        ):
            # ---- constants ----
            pos_t = [_T(c_pool, [P, P * (ci + 1)], BF16, f"pos{ci}")
                     for ci in range(NCS)]
            for ci in range(NCS):
                nc.sync.dma_start(pos_t[ci][:, :], posn[ci, :, :P * (ci + 1)])
            mkd = [_T(c_pool, [P, P], BF16, f"mkd{m}") for m in range(2)]
            mkinf = [_T(c_pool, [P, P], BF16, f"mkinf{m}") for m in range(2)]
            ninv_t = [_T(c_pool, [P, 1], F32, f"ninv{m}") for m in range(2)]
            eps_t = _T(c_pool, [P, 1], F32, "eps_t")
            nc.vector.memset(eps_t[:, :], 1e-30)
            for m in range(2):
                nc.sync.dma_start(mkd[m][:, :], maskd[m, :, :])
                nc.sync.dma_start(mkinf[m][:, :], maskinf[m, :, :])
                nc.sync.dma_start(ninv_t[m][:, :], ninv[m, :].unsqueeze(1))
            idn = _T(c_pool, [P, P], F32, "ident")
            nc.sync.dma_start(idn[:, :], ident[:, :])
            ones_f = _T(c_pool, [P, 1], F32, "ones_f")
            nc.vector.memset(ones_f[:, :], 1.0)
            ones_b = _T(c_pool, [P, 1], BF16, "ones_b")
            nc.vector.memset(ones_b[:, :], 1.0)
            g2t = {}
            for l in range(L):
                for h in range(H):
                    g2t[(l, h)] = _T(c_pool, [P, 1], F32, f"g2_{l}_{h}")
                    nc.sync.dma_start(g2t[(l, h)][:, :], g2n[l, h, :].unsqueeze(1))

            # ---- residual streams (feature-major, f32 + bf16 copy) ----
            xs = {}   # (stream, b) -> list of 4 tiles [128, S] f32
            xbf = {}  # bf16 copies
            for st, src in (("x", xT), ("y", yT)):
                for b in range(BL):
                    xs[(st, b)] = []
                    xbf[(st, b)] = []
                    for kc in range(NCD):
                        t = _T(a_pool, [P, S], F32, f"{st}{b}{kc}")
                        nc.sync.dma_start(t[:, :], src[b, kc * P:(kc + 1) * P, :])
                        tb = _T(abf_pool, [P, S], BF16, f"{st}b{b}{kc}")
                        nc.vector.tensor_copy(tb[:, :], t[:, :])
                        xs[(st, b)].append(t)
                        xbf[(st, b)].append(tb)

            # ---- weight loads per block (streamed) ----
            def load_w(l, ffn):
                wt = {}
                q = [0]

                def wload(t, srcap):
                    nc.sync.dma_start(t, srcap)

                for name, src, pl in (("wk", wk, wk_pool), ("wv", wv, wvo_pool),
                                      ("wo", wo, wvo_pool)):
                    wt[name] = []
                    for kc in range(NCD):
                        t = _T(pl, [P, D], BF16, f"{name}{kc}")
                        wload(t[:, :], src[l, kc * P:(kc + 1) * P, :])
                        wt[name].append(t)
                if ffn:
                    wt["w1"] = []
                    for kc in range(NCD):
                        t = _T(wff_pool, [P, FF], BF16, f"w1{kc}")
                        wload(t[:, :], w1[l, kc * P:(kc + 1) * P, :])
                        wt["w1"].append(t)
                    wt["w2"] = []
                    for fc in range(NCF):
                        t = _T(wff_pool, [P, D], BF16, f"w2{fc}")
                        wload(t[:, :], w2[l, fc * P:(fc + 1) * P, :])
                        wt["w2"].append(t)
                return wt

            # ---- residual add + layernorm (feature-major) ----
            # psum_tiles: 4 x [128,S] f32 (new contribution); prev: f32 tiles
            # writes new stream tiles (f32 + bf16) under the stream's tags
            def add_ln(b, st, psum_tiles, prev):
                xsum = prev
                for kc in range(NCD):
                    nc.vector.tensor_add(prev[kc][:, :], psum_tiles[kc][:, :],
                                         prev[kc][:, :])
                # stats: sum x (f32 mm), sum x^2 (bf16 mm on squared)
                st_ps = _T(psC, [33, S], F32, "ps_st")
                sq = []
                for kc in range(NCD):
                    q = _T(s_pool, [P, S], BF16, f"lnw{kc}")
                    nc.gpsimd.tensor_tensor(q[:, :], xsum[kc][:, :],
                                            xsum[kc][:, :], OP.mult)
                    sq.append(q)
                for kc in range(NCD):
                    nc.tensor.matmul(st_ps[0:1, :], ones_f[:, :], xsum[kc][:, :],
                                     start=(kc == 0), stop=(kc == NCD - 1))
                for kc in range(NCD):
                    nc.tensor.matmul(st_ps[32:33, :], ones_b[:, :], sq[kc][:, :],
                                     start=(kc == 0), stop=(kc == NCD - 1))
                mean = _T(s_pool, [1, S], F32, "mean")
                nc.vector.tensor_scalar(mean[:, :], st_ps[0:1, :], 1.0 / D, None,
                                        OP.mult)
                tmp = _T(s_pool, [1, S], F32, "lntmp")
                nc.vector.tensor_tensor(tmp[:, :], mean[:, :], mean[:, :], OP.mult)
                nc.vector.scalar_tensor_tensor(tmp[:, :], st_ps[32:33, :], 1.0 / D,
                                               tmp[:, :], OP.mult, OP.subtract)
                nc.scalar.activation(tmp[:, :], tmp[:, :], AF.Ln, bias=EPS)
                st2 = _T(s_pool, [1, 2 * S], BF16, "st2")
                nc.scalar.activation(st2[:, :S], tmp[:, :], AF.Exp,
                                     scale=-0.5)                     # rstd
                nc.vector.scalar_tensor_tensor(st2[:, S:], mean[:, :], -1.0,
                                               st2[:, :S], OP.mult, OP.mult)
                rnb = _T(s_pool, [P, 2 * S], BF16, "rnb")
                nc.gpsimd.partition_broadcast(rnb[:, :], st2[0:1, :])
                rstd_b = rnb[:, :S]
                nmr_b = rnb[:, S:]
                newxb = []
                for kc in range(NCD):
                    tmp = _T(s_pool, [P, S], F32, f"lnw{kc}")
                    nc.gpsimd.tensor_tensor(tmp[:, :], xsum[kc][:, :], rstd_b,
                                            OP.mult)
                    nc.gpsimd.tensor_tensor(prev[kc][:, :], tmp[:, :], nmr_b,
                                            OP.add)
                    ntb = _T(abf_pool, [P, S], BF16, f"{st}b{b}{kc}")
                    nc.vector.tensor_copy(ntb[:, :], prev[kc][:, :])
                    newxb.append(ntb)
                xbf[(st, b)] = newxb

            # ================= main blocks =================
            for l, (mt, ffn, qk_in, v_in) in list(enumerate(BLOCKS)) * repeat:
                wt = load_w(l, ffn)

                kq = {}   # (b) -> 4 tiles [128,S] bf16 feature-major
                vv = {}   # (b) -> 4 tiles [128,D] bf16 token-major
                for b in range(BL):
                    # kq projection (feature-major): lhsT=Wk[k,n] rhs=xT[k,:]
                    src = xbf[(qk_in, b)]
                    kq[b] = []
                    for nch in range(NCD):
                        ps = _T(psB, [P, S], F32, "ps_mm")
                        for kc in range(NCD):
                            nc.tensor.matmul(
                                ps[:, :], wt["wk"][kc][:, nch * P:(nch + 1) * P],
                                src[kc][:, :], start=(kc == 0), stop=(kc == NCD - 1))
                        t = _T(kqv_pool, [P, S], BF16, f"kq{b}{nch}")
                        act(t[:, :], ps[:, :], AF.Copy)
                        kq[b].append(t)
                    # v projection (token-major): lhsT=yT[k, j-chunk] rhs=Wv[k,:]
                    srcv = xbf[(v_in, b)]
                    vv[b] = []
                    for jc in range(NCS):
                        ps = _T(psB, [P, D], F32, "ps_mm")
                        for kc in range(NCD):
                            nc.tensor.matmul(
                                ps[:, :], srcv[kc][:, jc * P:(jc + 1) * P],
                                wt["wv"][kc][:, :], start=(kc == 0),
                                stop=(kc == NCD - 1))
                        t = _T(kqv_pool, [P, D], BF16, f"v{b}{jc}")
                        act(t[:, :], ps[:, :], AF.Copy)
                        vv[b].append(t)

                # ---- attention phases; C(b0) interleaves with A(b1) ----
                def phaseA_head(b, U, h):
                    hc, hr = h // 2, (h % 2) * DK
                    ut = _T(u_pool, [P, 1280], BF16, f"u{h}")
                    U[h] = ut
                    off = 0
                    for ci in range(NCS):
                        W = P * (ci + 1)
                        ps = psA.tile([P, S], F32, tag="ps_sc", name="ps_sc",
                                      bufs=4)
                        nc.tensor.matmul(
                            ps[:, :W],
                            kq[b][hc][hr:hr + DK, ci * P:(ci + 1) * P],
                            kq[b][hc][hr:hr + DK, :W], start=True, stop=True)
                        e1 = at_pool.tile([P, S], BF16, tag="e1", name="e1",
                                          bufs=3)
                        act(e1[:, :W], ps[:, :W], AF.Exp)
                        nc.vector.tensor_tensor(
                            e1[:, ci * P:W], e1[:, ci * P:W], mkd[mt][:, :],
                            OP.mult)
                        cum = at_pool.tile([P, S], F32, tag="cum", name="cum",
                                           bufs=2)
                        nc.vector.tensor_tensor_scan(
                            cum[:, :W], e1[:, :W], e1[:, :W], 0.0,
                            OP.add, OP.bypass)
                        rz = _T(s_pool, [P, 1], F32, "rz1")
                        nc.vector.reciprocal(rz[:, :], cum[:, W - 1:W])
                        tt = _T(at_pool, [P, S], BF16, "tt")
                        nc.vector.tensor_scalar(tt[:, :W], cum[:, :W],
                                                rz[:, :], 1.0, OP.mult, OP.min)
                        nc.vector.scalar_tensor_tensor(
                            ut[:, off:off + W], tt[:, :W], 1.0,
                            pos_t[ci][:, :W], OP.subtract, OP.mult)
                        off += W

                def phaseB(U):
                    for h in range(H):
                        act(U[h][:, :], U[h][:, :], AF.Sqrt,
                            scale=g2t[(l, h)][:, :])

                def mk_te(U, te_tiles, h):
                    t_ = at_pool.tile([P, 1280], BF16, tag="te", name="te",
                                      bufs=3)
                    act(t_[:, :], U[h][:, :], AF.Exp, scale=-1.0)
                    te_tiles[h] = t_

                def phaseC_head(b, U, te_tiles, ot, h):
                    if h == 0:
                        mk_te(U, te_tiles, 0)
                        mk_te(U, te_tiles, 1)
                    elif h + 1 < H:
                        mk_te(U, te_tiles, h + 1)
                    hc, hr = h // 2, (h % 2) * DK
                    te = te_tiles.pop(h)
                    e2ta = _T(at_pool, [P, NCS, S], BF16, "e2ta")
                    z2 = _T(s_pool, [P, NCS], F32, "z2")
                    off = 0
                    for ci in range(NCS):
                        W = P * (ci + 1)
                        ps = psA.tile([P, S], F32, tag="ps_sc", name="ps_sc",
                                      bufs=4)
                        nc.tensor.matmul(
                            ps[:, :W],
                            kq[b][hc][hr:hr + DK, ci * P:(ci + 1) * P],
                            kq[b][hc][hr:hr + DK, :W], start=True, stop=True)
                        nc.vector.tensor_tensor(ps[:, :W], te[:, off:off + W],
                                                ps[:, :W], OP.mult)
                        nc.vector.tensor_tensor(
                            ps[:, ci * P:W], ps[:, ci * P:W], mkinf[mt][:, :],
                            OP.add)
                        e2 = at_pool.tile([P, S], BF16, tag="e2", name="e2",
                                          bufs=2)
                        act(e2[:, :W], ps[:, :W], AF.Exp,
                            accum_out=z2[:, ci:ci + 1])
                        rz = _T(s_pool, [P, 1], F32, "rz2")
                        nc.vector.tensor_scalar(rz[:, :], z2[:, ci:ci + 1],
                                                1e-30, None, OP.add)
                        nc.vector.reciprocal(rz[:, :], rz[:, :])
                        e2n = at_pool.tile([P, S], BF16, tag="e2n", name="e2n",
                                           bufs=3)
                        nc.vector.tensor_scalar(e2n[:, :W], e2[:, :W],
                                                rz[:, :], None, OP.mult)
                        eng = nc.sync if (h + ci) % 2 == 0 else nc.scalar
                        eng.dma_start_transpose(
                            e2ta[:, 0:ci + 1, ci * P:(ci + 1) * P],
                            e2n[:, :W])
                        off += W
                    # av: oT[d, i] = sum_j v[j,d] * e2T[j,i]
                    po = _T(psC, [DK, S], F32, "ps_o")
                    for cj in range(NCS):
                        nc.tensor.matmul(
                            po[:, cj * P:], vv[b][cj][:, h * DK:(h + 1) * DK],
                            e2ta[:, cj, cj * P:], start=(cj == 0),
                            stop=(cj == NCS - 1), skip_group_check=True)
                    osl = ot[h // 2][(h % 2) * DK:(h % 2 + 1) * DK, :]
                    if mt == 1:
                        # mask0 row 0: attn = uniform -> o[0] = mean_j v[j]
                        pm = _T(psC, [DK, 1], F32, "ps_st")
                        for cj in range(NCS):
                            nc.tensor.matmul(
                                pm[:, :], vv[b][cj][:, h * DK:(h + 1) * DK],
                                ones_b[:, :], start=(cj == 0),
                                stop=(cj == NCS - 1))
                        if h % 2 == 0:
                            act(osl[:, 1:], po[:, 1:], AF.Copy)
                        else:
                            nc.vector.tensor_copy(osl[:, 1:], po[:, 1:])
                        act(osl[:, 0:1], pm[:, :], AF.Copy, scale=1.0 / S)
                    else:
                        if h % 2 == 0:
                            act(osl, po[:, :], AF.Copy)
                        else:
                            nc.vector.tensor_copy(osl, po[:, :])

                def post_b(b, ot):
                    pss = []
                    for nch in range(NCD):
                        ps = _T(psB, [P, S], F32, "ps_mm")
                        for dc in range(NCD):
                            nc.tensor.matmul(
                                ps[:, :], wt["wo"][dc][:, nch * P:(nch + 1) * P],
                                ot[dc][:, :], start=(dc == 0),
                                stop=(dc == NCD - 1))
                        pss.append(ps)
                    add_ln(b, qk_in, pss, xs[(qk_in, b)])
                    if ffn:
                        srcf = xbf[(qk_in, b)]
                        hts = []
                        for fc in range(NCF):
                            ps = _T(psB, [P, S], F32, "ps_mm")
                            for kc in range(NCD):
                                nc.tensor.matmul(
                                    ps[:, :], wt["w1"][kc][:, fc * P:(fc + 1) * P],
                                    srcf[kc][:, :], start=(kc == 0),
                                    stop=(kc == NCD - 1))
                            ht = _T(h_pool, [P, S], BF16, f"h{fc}")
                            if fc % 2 == 0:
                                act(ht[:, :], ps[:, :], AF.Relu)
                            else:
                                nc.vector.tensor_scalar_max(ht[:, :], ps[:, :],
                                                            0.0)
                            hts.append(ht)
                        pss = []
                        for nch in range(NCD):
                            ps = _T(psB, [P, S], F32, "ps_mm")
                            for fc in range(NCF):
                                nc.tensor.matmul(
                                    ps[:, :], wt["w2"][fc][:, nch * P:(nch + 1) * P],
                                    hts[fc][:, :], start=(fc == 0),
                                    stop=(fc == NCF - 1))
                            pss.append(ps)
                        add_ln(b, qk_in, pss, xs[(qk_in, b)])

                U0, U1 = {}, {}
                ot0 = [_T(kqv_pool, [P, S], BF16, f"ot0{dc}")
                       for dc in range(NCD)]
                ot1 = [_T(kqv_pool, [P, S], BF16, f"ot1{dc}")
                       for dc in range(NCD)]
                for h in range(H):
                    phaseA_head(0, U0, h)
                phaseB(U0)
                te0, te1 = {}, {}
                for h in range(H):
                    phaseC_head(0, U0, te0, ot0, h)
                post_b(0, ot0)
                for h in range(H):
                    phaseA_head(1, U1, h)
                phaseB(U1)
                for h in range(H):
                    phaseC_head(1, U1, te1, ot1, h)
                post_b(1, ot1)

            # ---- final transpose to token-major + store ----
            for b in range(BL):
                for tcn in range(NCS):
                    xo = _T(s_pool, [P, D], F32, "lnw0")
                    for nch in range(NCD):
                        pt = _T(psC, [P, P], F32, "ps_st")
                        nc.tensor.transpose(
                            pt[:, :], xs[("x", b)][nch][:, tcn * P:(tcn + 1) * P],
                            idn[:, :])
                        act(xo[:, nch * P:(nch + 1) * P], pt[:, :], AF.Copy)
                    nc.sync.dma_start(out[b, tcn * P:(tcn + 1) * P, :], xo[:, :])

    nc.finalize()
    return nc


def _prep_inputs(inputs):
    """Host-side preprocessing -> per-core in_maps."""
    f32 = np.float32
    bf = ml_dtypes.bfloat16
    q = np.asarray(inputs["q_embed_data"], f32)
    qa = np.asarray(inputs["qa_embed_data"], f32)
    Wk = np.asarray(inputs["Wk"], f32)
    Wv = np.asarray(inputs["Wv"], f32)
    Wo = np.asarray(inputs["Wo"], f32)
    W1 = np.asarray(inputs["W1"], f32)
    W2 = np.asarray(inputs["W2"], f32)
    gam = np.asarray(inputs["gammas"], f32).reshape(L, H)

    scale = 1.0 / math.sqrt(math.sqrt(DK))
    wk_b = (Wk * scale).astype(bf)
    wv_b = Wv.astype(bf)
    wo_b = Wo.astype(bf)
    w1_b = W1.astype(bf)
    w2_b = W2.astype(bf)

    sp = _softplus(gam)
    g2n = np.repeat((-(sp ** 2))[:, :, None], P, axis=2).astype(f32)

    jj = np.arange(S)[None, :]
    posn = np.zeros((NCS, P, S), np.float64)
    for ci in range(NCS):
        ii = (ci * P + np.arange(P))[:, None]
        posn[ci] = np.abs(ii - jj)
    posn = posn.astype(bf)

    r = np.arange(P)
    maskd = np.zeros((2, P, P), np.float64)
    maskd[0] = (r[:, None] >= r[None, :])
    maskd[1] = (r[:, None] > r[None, :])
    maskinf = np.where(maskd > 0, 0.0, -np.inf).astype(bf)
    ninv = np.stack([P - 1.0 - r, np.float64(P) - r]).astype(f32)
    maskd = maskd.astype(bf)

    ident = np.eye(P, dtype=f32)

    in_maps = []
    for c in range(NCORES):
        sl = slice(c * BL, (c + 1) * BL)
        xTc = np.ascontiguousarray(q[sl].transpose(0, 2, 1))
        yTc = np.ascontiguousarray(qa[sl].transpose(0, 2, 1))
        in_maps.append(dict(
            xT=xTc, yT=yTc, wk=wk_b, wv=wv_b, wo=wo_b, w1=w1_b, w2=w2_b,
            g2n=g2n, posn=posn, maskd=maskd, maskinf=maskinf, ninv=ninv, ident=ident,
        ))
    return in_maps


def kernel(**inputs):
    if "nc" not in _CACHE:
        _CACHE["nc"] = build_graph()
    nc = _CACHE["nc"]
    in_maps = _prep_inputs(inputs)
    res = run_bass_kernel_spmd(nc, in_maps, core_ids=list(range(NCORES)))
    outs = [r["out"] for r in res.results]
    return np.concatenate(outs, axis=0).astype(np.float32)

